# revision 4
# baseline (speedup 1.0000x reference)
"""EnsembleTransitionMLP Trainium2 kernel.

Problem: 50-member ensemble of 4-layer MLPs (40 -> 256 -> 256 -> 256 -> 33),
shared input batch [8192, 40], fp32.

Sharding (8 cores): hybrid expert+batch. Work = 50 members x 4 batch-chunks
of 2048 = 200 units; each core gets 25 units: 6 full members (4 chunks each)
plus one quarter of a "leftover" member (members 48/49 are split 4-ways by
batch). Every core runs an identical instruction stream; per-core data
(weight slots, leftover batch slice) differs only in the input maps.

On-chip mapping: activations live as H^T [hidden on partitions, batch on
free], so weights load directly as lhsT=[K, M] (natural [in, out] layout)
and no transposes are ever needed. Host transposes SA once and the final
output once.

Matmul dtype: float32r (full fp32 bits; PE runs it at 1 cycle/row for
moving-dim >= 256, vs 4 cycles/row for plain fp32).
"""

import os
import sys
from contextlib import ExitStack

import numpy as np

import concourse.bass as bass
import concourse.tile as tile
from concourse import bacc, mybir
from concourse.bass_utils import run_bass_kernel_spmd

# ---------------------------------------------------------------- constants
CORES = 8
E = 50
B = 8192
IN_DIM = 40  # state 32 + action 8
H = 256
OD = 33  # next_state 32 + reward 1
SLOTS = 7  # per-core weight slots: 6 full members + 1 leftover member
CHUNK = 2048  # batch columns per unit
NT = CHUNK // 512  # 512-column N-tiles per unit
UNITS = 25  # 24 regular (slot u//4, chunk u%4) + 1 leftover (slot 6)

F32 = mybir.dt.float32
F32R = mybir.dt.float32r

# 'f32r' = full-rate fp32 | 'f32' = exact but 4x slower PE | 'bf16'
MM_MODE = os.environ.get("MLP_MM_MODE", "f32r")


def _mm(ap):
    return ap


# ---------------------------------------------------------------- program
def build_program():
    nc = bacc.Bacc(
        "TRN2",
        target_bir_lowering=False,
        debug=False,
        num_devices=CORES,
    )
    if MM_MODE == "bf16":
        mmdt = mybir.dt.bfloat16
    elif MM_MODE == "f32r":
        mmdt = F32R
    else:
        mmdt = F32

    sat_d = nc.dram_tensor("sat", [IN_DIM, B], mmdt, kind="ExternalInput").ap()
    satx_d = nc.dram_tensor("satx", [IN_DIM, CHUNK], mmdt, kind="ExternalInput").ap()
    w1_d = nc.dram_tensor("w1", [SLOTS, IN_DIM, H], mmdt, kind="ExternalInput").ap()
    w2_d = nc.dram_tensor("w2", [SLOTS, H, H], mmdt, kind="ExternalInput").ap()
    w3_d = nc.dram_tensor("w3", [SLOTS, H, H], mmdt, kind="ExternalInput").ap()
    w4_d = nc.dram_tensor("w4", [SLOTS, H, OD], mmdt, kind="ExternalInput").ap()
    b1_d = nc.dram_tensor("b1", [SLOTS, H], F32, kind="ExternalInput").ap()
    b2_d = nc.dram_tensor("b2", [SLOTS, H], F32, kind="ExternalInput").ap()
    b3_d = nc.dram_tensor("b3", [SLOTS, H], F32, kind="ExternalInput").ap()
    b4_d = nc.dram_tensor("b4", [SLOTS, OD], F32, kind="ExternalInput").ap()
    out_d = nc.dram_tensor("out", [UNITS, OD, CHUNK], F32, kind="ExternalOutput").ap()

    with tile.TileContext(nc) as tc, ExitStack() as ctx:
        wpool = ctx.enter_context(tc.tile_pool(name="wpool", bufs=1))
        spool = ctx.enter_context(tc.tile_pool(name="spool", bufs=1))
        hpool = ctx.enter_context(tc.tile_pool(name="hpool", bufs=1))
        opool = ctx.enter_context(tc.tile_pool(name="opool", bufs=3))
        ppool = ctx.enter_context(tc.tile_pool(name="ppool", bufs=6, space="PSUM"))

        # ---- resident inputs -------------------------------------------
        sat_t = spool.tile([IN_DIM, B], mmdt, name="sat_t")
        for c in range(4):  # chunked so unit 0 waits only on its own slice
            nc.sync.dma_start(
                out=sat_t[:, c * CHUNK : (c + 1) * CHUNK],
                in_=sat_d[:, c * CHUNK : (c + 1) * CHUNK],
            )
        satx_t = spool.tile([IN_DIM, CHUNK], mmdt, name="satx_t")
        nc.sync.dma_start(out=satx_t[:, :], in_=satx_d[:, :])

        # ---- resident weights: slot s -> tiles -------------------------
        w1t, w2t, w3t, w4t = [], [], [], []
        b1t, b2t, b3t, b4t = [], [], [], []
        for s in range(SLOTS):
            w1 = wpool.tile([IN_DIM, H], mmdt, name=f"w1_{s}", tag=f"w1_{s}")
            nc.sync.dma_start(out=w1[:, :], in_=w1_d[s, :, :])
            w1t.append(w1)

            # [K=256 -> 2 chunks of 128 rows] stored side by side in free dim
            w2 = wpool.tile([128, 2 * H], mmdt, name=f"w2_{s}", tag=f"w2_{s}")
            for k in range(2):
                nc.sync.dma_start(
                    out=w2[:, k * H : (k + 1) * H],
                    in_=w2_d[s, k * 128 : (k + 1) * 128, :],
                )
            w2t.append(w2)

            w3 = wpool.tile([128, 2 * H], mmdt, name=f"w3_{s}", tag=f"w3_{s}")
            for k in range(2):
                nc.sync.dma_start(
                    out=w3[:, k * H : (k + 1) * H],
                    in_=w3_d[s, k * 128 : (k + 1) * 128, :],
                )
            w3t.append(w3)

            w4 = wpool.tile([128, 2 * OD], mmdt, name=f"w4_{s}", tag=f"w4_{s}")
            for k in range(2):
                nc.sync.dma_start(
                    out=w4[:, k * OD : (k + 1) * OD],
                    in_=w4_d[s, k * 128 : (k + 1) * 128, :],
                )
            w4t.append(w4)

            b1 = wpool.tile([128, 2], F32, name=f"b1_{s}", tag=f"b1_{s}")
            b2 = wpool.tile([128, 2], F32, name=f"b2_{s}", tag=f"b2_{s}")
            b3 = wpool.tile([128, 2], F32, name=f"b3_{s}", tag=f"b3_{s}")
            for m in range(2):
                nc.sync.dma_start(out=b1[:, m : m + 1], in_=b1_d[s, m * 128 : (m + 1) * 128])
                nc.sync.dma_start(out=b2[:, m : m + 1], in_=b2_d[s, m * 128 : (m + 1) * 128])
                nc.sync.dma_start(out=b3[:, m : m + 1], in_=b3_d[s, m * 128 : (m + 1) * 128])
            b4 = wpool.tile([OD, 1], F32, name=f"b4_{s}", tag=f"b4_{s}")
            nc.sync.dma_start(out=b4[:, :], in_=b4_d[s, :])
            b1t.append(b1)
            b2t.append(b2)
            b3t.append(b3)
            b4t.append(b4)

        # ---- fused bias + relu (PSUM -> SBUF), alternating ACT / DVE ---
        eng_flip = [0]

        def bias_relu(dst, src, bias_ap, relu):
            e = eng_flip[0]
            eng_flip[0] ^= 1
            if e == 0:
                func = (
                    mybir.ActivationFunctionType.Relu
                    if relu
                    else mybir.ActivationFunctionType.Identity
                )
                nc.scalar.activation(dst, src, func, bias=bias_ap, scale=1.0)
            else:
                if relu:
                    nc.vector.tensor_scalar(
                        dst, src, bias_ap, 0.0, mybir.AluOpType.add, mybir.AluOpType.max
                    )
                else:
                    nc.vector.tensor_scalar(
                        dst, src, bias_ap, None, mybir.AluOpType.add
                    )

        # ---- compute: 25 units -----------------------------------------
        for u in range(UNITS):
            s = u // 4 if u < 24 else 6
            src = sat_t if u < 24 else satx_t
            c0 = (u % 4) * CHUNK if u < 24 else 0

            # L1: h1[mtile][:, n] = relu(W1[:, mtile]^T @ sa^T + b1)
            h1 = hpool.tile([128, 2, CHUNK], mmdt, name="h1", tag="h1")
            for t in range(NT):
                nsl = slice(t * 512, (t + 1) * 512)
                for m in range(2):
                    ps = ppool.tile([128, 512], F32, name="ps", tag="ps")
                    nc.tensor.matmul(
                        out=ps[:, :],
                        lhsT=_mm(w1t[s][:, m * 128 : (m + 1) * 128]),
                        rhs=_mm(src[:, c0 + t * 512 : c0 + (t + 1) * 512]),
                        start=True,
                        stop=True,
                    )
                    bias_relu(h1[:, m, nsl], ps[:, :], b1t[s][:, m : m + 1], True)

            # L2 / L3: h_next[m] = relu(sum_k W[k][:, m]^T @ h_prev[k] + b)
            h_prev = h1
            for w, bt, nm in ((w2t, b2t, "h2"), (w3t, b3t, "h3")):
                h_nxt = hpool.tile([128, 2, CHUNK], mmdt, name=nm, tag=nm)
                for t in range(NT):
                    nsl = slice(t * 512, (t + 1) * 512)
                    for m in range(2):
                        ps = ppool.tile([128, 512], F32, name="ps", tag="ps")
                        for k in range(2):
                            nc.tensor.matmul(
                                out=ps[:, :],
                                lhsT=_mm(w[s][:, k * H + m * 128 : k * H + (m + 1) * 128]),
                                rhs=_mm(h_prev[:, k, nsl]),
                                start=(k == 0),
                                stop=(k == 1),
                            )
                        bias_relu(h_nxt[:, m, nsl], ps[:, :], bt[s][:, m : m + 1], True)
                h_prev = h_nxt

            # L4: out[u] = W4^T @ h3 + b4   (no relu)
            ot = opool.tile([OD, CHUNK], F32, name="ot", tag="ot")
            for t in range(NT):
                nsl = slice(t * 512, (t + 1) * 512)
                ps = ppool.tile([128, 512], F32, name="ps", tag="ps")
                for k in range(2):
                    nc.tensor.matmul(
                        out=ps[:OD, :],
                        lhsT=_mm(w4t[s][:, k * OD : (k + 1) * OD]),
                        rhs=_mm(h_prev[:, k, nsl]),
                        start=(k == 0),
                        stop=(k == 1),
                    )
                bias_relu(ot[:, nsl], ps[:OD, :], b4t[s][:, 0:1], False)
            nc.sync.dma_start(out=out_d[u, :, :], in_=ot[:, :])

    nc.compile()
    return nc


# ---------------------------------------------------------------- host side
def _cast(a):
    if MM_MODE == "bf16":
        import ml_dtypes

        return np.asarray(a, dtype=ml_dtypes.bfloat16)
    return np.ascontiguousarray(a, dtype=np.float32)


def make_in_maps(inputs):
    sa = np.concatenate(
        [np.asarray(inputs["state"]), np.asarray(inputs["action"])], axis=1
    )
    sat = _cast(sa.T)  # [40, 8192]
    in_maps = []
    for k in range(CORES):
        members = list(range(k * 6, (k + 1) * 6)) + [48 + k // 4]
        lc = k % 4  # leftover member's batch chunk handled by this core
        im = {
            "sat": sat,
            "satx": _cast(sat[:, lc * CHUNK : (lc + 1) * CHUNK]),
            "w1": _cast(np.asarray(inputs["W1"])[members]),
            "w2": _cast(np.asarray(inputs["W2"])[members]),
            "w3": _cast(np.asarray(inputs["W3"])[members]),
            "w4": _cast(np.asarray(inputs["W4"])[members]),
            "b1": np.ascontiguousarray(np.asarray(inputs["b1"])[members], np.float32),
            "b2": np.ascontiguousarray(np.asarray(inputs["b2"])[members], np.float32),
            "b3": np.ascontiguousarray(np.asarray(inputs["b3"])[members], np.float32),
            "b4": np.ascontiguousarray(np.asarray(inputs["b4"])[members], np.float32),
        }
        in_maps.append(im)
    return in_maps


def assemble(results):
    predsT = np.empty((E, OD, B), np.float32)
    for k in range(CORES):
        o = results[k]["out"]  # [25, 33, 2048]
        for u in range(24):
            s, c = divmod(u, 4)
            predsT[k * 6 + s, :, c * CHUNK : (c + 1) * CHUNK] = o[u]
        m = 48 + k // 4
        c = k % 4
        predsT[m, :, c * CHUNK : (c + 1) * CHUNK] = o[24]
    preds = predsT.transpose(2, 0, 1)  # [B, E, 33]
    return np.ascontiguousarray(preds[..., :-1]), np.ascontiguousarray(preds[..., -1])


_NC = None


def _get_nc():
    global _NC
    if _NC is None:
        _NC = build_program()
    return _NC


def run(inputs, trace=False, **kw):
    nc = _get_nc()
    in_maps = make_in_maps(inputs)
    res = run_bass_kernel_spmd(nc, in_maps, list(range(CORES)), trace=trace, **kw)
    return assemble(res.results), res


def kernel(**inputs):
    (next_state, reward), _ = run(inputs, trace=False)
    return next_state, reward


# revision 9
# speedup vs baseline: 1.0166x; 1.0166x over previous
"""EnsembleTransitionMLP Trainium2 kernel.

Problem: 50-member ensemble of 4-layer MLPs (40 -> 256 -> 256 -> 256 -> 33),
shared input batch [8192, 40], fp32.

Sharding (8 cores): hybrid expert+batch. Work = 50 members x 4 batch-chunks
of 2048 = 200 units; each core gets 25 units: 6 full members (4 chunks each)
plus one quarter of a "leftover" member (members 48/49 are split 4-ways by
batch). Every core runs an identical instruction stream; per-core data
(weight slots, leftover batch slice) differs only in the input maps.

On-chip mapping: activations live as H^T [hidden on partitions, batch on
free], so weights load directly as lhsT=[K, M] (natural [in, out] layout)
and no transposes are ever needed. Host transposes SA once and the final
output once.

Matmul dtype: float32r (full fp32 bits; PE runs it at 1 cycle/row for
moving-dim >= 256, vs 4 cycles/row for plain fp32).
"""

import os
import sys
from contextlib import ExitStack

import numpy as np

import concourse.bass as bass
import concourse.tile as tile
from concourse import bacc, mybir
from concourse.bass_utils import run_bass_kernel_spmd

# ---------------------------------------------------------------- constants
CORES = 8
E = 50
B = 8192
IN_DIM = 40  # state 32 + action 8
H = 256
OD = 33  # next_state 32 + reward 1
SLOTS = 7  # per-core weight slots: 6 full members + 1 leftover member
CHUNK = 2048  # batch columns per unit
NT = CHUNK // 512  # 512-column N-tiles per unit
UNITS = 25  # 24 regular (slot u//4, chunk u%4) + 1 leftover (slot 6)

F32 = mybir.dt.float32
F32R = mybir.dt.float32r

# 'f16' = fp16 in / fp32 accumulate (10-bit mantissa, FWL weight loads)
# 'f32r' = TF32 (10-bit mantissa, slow fp32 weight loads)
# 'f32'  = exact fp32, 4x slower PE | 'bf16' = fastest loads, 8-bit mantissa
MM_MODE = os.environ.get("MLP_MM_MODE", "f16")


def _mm(ap):
    return ap


# ---------------------------------------------------------------- program
def build_program():
    nc = bacc.Bacc(
        "TRN2",
        target_bir_lowering=False,
        debug=False,
        num_devices=CORES,
    )
    if MM_MODE == "bf16":
        mmdt = mybir.dt.bfloat16
    elif MM_MODE == "f16":
        mmdt = mybir.dt.float16
    elif MM_MODE == "f32r":
        mmdt = F32R
    else:
        mmdt = F32

    sat_d = nc.dram_tensor("sat", [IN_DIM, B], mmdt, kind="ExternalInput").ap()
    satx_d = nc.dram_tensor("satx", [IN_DIM, CHUNK], mmdt, kind="ExternalInput").ap()
    w1_d = nc.dram_tensor("w1", [SLOTS, IN_DIM, H], mmdt, kind="ExternalInput").ap()
    w2_d = nc.dram_tensor("w2", [SLOTS, H, H], mmdt, kind="ExternalInput").ap()
    w3_d = nc.dram_tensor("w3", [SLOTS, H, H], mmdt, kind="ExternalInput").ap()
    w4_d = nc.dram_tensor("w4", [SLOTS, H, OD], mmdt, kind="ExternalInput").ap()
    b1_d = nc.dram_tensor("b1", [SLOTS, H], F32, kind="ExternalInput").ap()
    b2_d = nc.dram_tensor("b2", [SLOTS, H], F32, kind="ExternalInput").ap()
    b3_d = nc.dram_tensor("b3", [SLOTS, H], F32, kind="ExternalInput").ap()
    b4_d = nc.dram_tensor("b4", [SLOTS, OD], F32, kind="ExternalInput").ap()
    out_d = nc.dram_tensor("out", [UNITS, OD, CHUNK], F32, kind="ExternalOutput").ap()

    with tile.TileContext(nc) as tc, ExitStack() as ctx:
        wpool = ctx.enter_context(tc.tile_pool(name="wpool", bufs=1))
        spool = ctx.enter_context(tc.tile_pool(name="spool", bufs=1))
        hpool = ctx.enter_context(tc.tile_pool(name="hpool", bufs=1))
        opool = ctx.enter_context(tc.tile_pool(name="opool", bufs=3))
        ppool = ctx.enter_context(tc.tile_pool(name="ppool", bufs=4, space="PSUM"))

        # ---- resident inputs -------------------------------------------
        sat_t = spool.tile([IN_DIM, B], mmdt, name="sat_t")
        for c in range(4):  # chunked so unit 0 waits only on its own slice
            nc.sync.dma_start(
                out=sat_t[:, c * CHUNK : (c + 1) * CHUNK],
                in_=sat_d[:, c * CHUNK : (c + 1) * CHUNK],
            )
        satx_t = spool.tile([IN_DIM, CHUNK], mmdt, name="satx_t")
        nc.sync.dma_start(out=satx_t[:, :], in_=satx_d[:, :])

        # ---- resident weights: slot s -> tiles -------------------------
        w1t, w2t, w3t, w4t = [], [], [], []
        b1t, b2t, b3t, b4t = [], [], [], []
        for s in range(SLOTS):
            w1 = wpool.tile([IN_DIM, H], mmdt, name=f"w1_{s}", tag=f"w1_{s}")
            nc.sync.dma_start(out=w1[:, :], in_=w1_d[s, :, :])
            w1t.append(w1)

            # [K=256 -> 2 chunks of 128 rows] stored side by side in free dim
            w2 = wpool.tile([128, 2 * H], mmdt, name=f"w2_{s}", tag=f"w2_{s}")
            for k in range(2):
                nc.sync.dma_start(
                    out=w2[:, k * H : (k + 1) * H],
                    in_=w2_d[s, k * 128 : (k + 1) * 128, :],
                )
            w2t.append(w2)

            w3 = wpool.tile([128, 2 * H], mmdt, name=f"w3_{s}", tag=f"w3_{s}")
            for k in range(2):
                nc.sync.dma_start(
                    out=w3[:, k * H : (k + 1) * H],
                    in_=w3_d[s, k * 128 : (k + 1) * 128, :],
                )
            w3t.append(w3)

            w4 = wpool.tile([128, 2 * OD], mmdt, name=f"w4_{s}", tag=f"w4_{s}")
            for k in range(2):
                nc.sync.dma_start(
                    out=w4[:, k * OD : (k + 1) * OD],
                    in_=w4_d[s, k * 128 : (k + 1) * 128, :],
                )
            w4t.append(w4)

            b1 = wpool.tile([128, 2], F32, name=f"b1_{s}", tag=f"b1_{s}")
            b2 = wpool.tile([128, 2], F32, name=f"b2_{s}", tag=f"b2_{s}")
            b3 = wpool.tile([128, 2], F32, name=f"b3_{s}", tag=f"b3_{s}")
            for m in range(2):
                nc.sync.dma_start(out=b1[:, m : m + 1], in_=b1_d[s, m * 128 : (m + 1) * 128])
                nc.sync.dma_start(out=b2[:, m : m + 1], in_=b2_d[s, m * 128 : (m + 1) * 128])
                nc.sync.dma_start(out=b3[:, m : m + 1], in_=b3_d[s, m * 128 : (m + 1) * 128])
            b4 = wpool.tile([OD, 1], F32, name=f"b4_{s}", tag=f"b4_{s}")
            nc.sync.dma_start(out=b4[:, :], in_=b4_d[s, :])
            b1t.append(b1)
            b2t.append(b2)
            b3t.append(b3)
            b4t.append(b4)

        # ---- fused bias + relu (PSUM -> SBUF), alternating ACT / DVE ---
        eng_flip = [0]

        def bias_relu(dst, src, bias_ap, relu):
            e = eng_flip[0]
            eng_flip[0] ^= 1
            if e == 0:
                func = (
                    mybir.ActivationFunctionType.Relu
                    if relu
                    else mybir.ActivationFunctionType.Identity
                )
                nc.scalar.activation(dst, src, func, bias=bias_ap, scale=1.0)
            else:
                if relu:
                    nc.vector.tensor_scalar(
                        dst, src, bias_ap, 0.0, mybir.AluOpType.add, mybir.AluOpType.max
                    )
                else:
                    nc.vector.tensor_scalar(
                        dst, src, bias_ap, None, mybir.AluOpType.add
                    )

        # ---- compute: 25 units -----------------------------------------
        # PSUM tiles are [128, 1024] (2 banks); matmuls fill 512-col halves
        # (each within one bank), bias+relu drains 1024 cols per op.
        for u in range(UNITS):
            s = u // 4 if u < 24 else 6
            src = sat_t if u < 24 else satx_t
            c0 = (u % 4) * CHUNK if u < 24 else 0

            # L1: h1[mtile][:, n] = relu(W1[:, mtile]^T @ sa^T + b1)
            h1 = hpool.tile([128, 2, CHUNK], mmdt, name="h1", tag="h1")
            for tp in range(NT // 2):  # 1024-col pair
                for m in range(2):
                    ps = ppool.tile([128, 1024], F32, name="ps", tag="ps")
                    for th in range(2):
                        t = 2 * tp + th
                        nc.tensor.matmul(
                            out=ps[:, th * 512 : (th + 1) * 512],
                            lhsT=w1t[s][:, m * 128 : (m + 1) * 128],
                            rhs=src[:, c0 + t * 512 : c0 + (t + 1) * 512],
                            start=True,
                            stop=True,
                        )
                    bias_relu(
                        h1[:, m, tp * 1024 : (tp + 1) * 1024],
                        ps[:, :],
                        b1t[s][:, m : m + 1],
                        True,
                    )

            # L2 / L3: h_next[m] = relu(sum_k W[k][:, m]^T @ h_prev[k] + b)
            h_prev = h1
            for w, bt, nm in ((w2t, b2t, "h2"), (w3t, b3t, "h3")):
                h_nxt = hpool.tile([128, 2, CHUNK], mmdt, name=nm, tag=nm)
                for tp in range(NT // 2):
                    for m in range(2):
                        ps = ppool.tile([128, 1024], F32, name="ps", tag="ps")
                        for th in range(2):
                            t = 2 * tp + th
                            nsl = slice(t * 512, (t + 1) * 512)
                            for k in range(2):
                                nc.tensor.matmul(
                                    out=ps[:, th * 512 : (th + 1) * 512],
                                    lhsT=w[s][:, k * H + m * 128 : k * H + (m + 1) * 128],
                                    rhs=h_prev[:, k, nsl],
                                    start=(k == 0),
                                    stop=(k == 1),
                                )
                        bias_relu(
                            h_nxt[:, m, tp * 1024 : (tp + 1) * 1024],
                            ps[:, :],
                            bt[s][:, m : m + 1],
                            True,
                        )
                h_prev = h_nxt

            # L4: out[u] = W4^T @ h3 + b4   (no relu)
            ot = opool.tile([OD, CHUNK], F32, name="ot", tag="ot")
            for tp in range(NT // 2):
                ps = ppool.tile([128, 1024], F32, name="ps", tag="ps")
                for th in range(2):
                    t = 2 * tp + th
                    nsl = slice(t * 512, (t + 1) * 512)
                    for k in range(2):
                        nc.tensor.matmul(
                            out=ps[:OD, th * 512 : (th + 1) * 512],
                            lhsT=w4t[s][:, k * OD : (k + 1) * OD],
                            rhs=h_prev[:, k, nsl],
                            start=(k == 0),
                            stop=(k == 1),
                        )
                bias_relu(
                    ot[:, tp * 1024 : (tp + 1) * 1024],
                    ps[:OD, :],
                    b4t[s][:, 0:1],
                    False,
                )
            nc.sync.dma_start(out=out_d[u, :, :], in_=ot[:, :])

    nc.compile()
    return nc


# ---------------------------------------------------------------- host side
def _cast(a):
    if MM_MODE == "bf16":
        import ml_dtypes

        return np.asarray(a, dtype=ml_dtypes.bfloat16)
    if MM_MODE == "f16":
        return np.ascontiguousarray(np.asarray(a, dtype=np.float32).astype(np.float16))
    return np.ascontiguousarray(a, dtype=np.float32)


def make_in_maps(inputs):
    sa = np.concatenate(
        [np.asarray(inputs["state"]), np.asarray(inputs["action"])], axis=1
    )
    sat = _cast(sa.T)  # [40, 8192]
    in_maps = []
    for k in range(CORES):
        members = list(range(k * 6, (k + 1) * 6)) + [48 + k // 4]
        lc = k % 4  # leftover member's batch chunk handled by this core
        im = {
            "sat": sat,
            "satx": _cast(sat[:, lc * CHUNK : (lc + 1) * CHUNK]),
            "w1": _cast(np.asarray(inputs["W1"])[members]),
            "w2": _cast(np.asarray(inputs["W2"])[members]),
            "w3": _cast(np.asarray(inputs["W3"])[members]),
            "w4": _cast(np.asarray(inputs["W4"])[members]),
            "b1": np.ascontiguousarray(np.asarray(inputs["b1"])[members], np.float32),
            "b2": np.ascontiguousarray(np.asarray(inputs["b2"])[members], np.float32),
            "b3": np.ascontiguousarray(np.asarray(inputs["b3"])[members], np.float32),
            "b4": np.ascontiguousarray(np.asarray(inputs["b4"])[members], np.float32),
        }
        in_maps.append(im)
    return in_maps


def assemble(results):
    predsT = np.empty((E, OD, B), np.float32)
    for k in range(CORES):
        o = results[k]["out"]  # [25, 33, 2048]
        for u in range(24):
            s, c = divmod(u, 4)
            predsT[k * 6 + s, :, c * CHUNK : (c + 1) * CHUNK] = o[u]
        m = 48 + k // 4
        c = k % 4
        predsT[m, :, c * CHUNK : (c + 1) * CHUNK] = o[24]
    preds = predsT.transpose(2, 0, 1)  # [B, E, 33]
    return np.ascontiguousarray(preds[..., :-1]), np.ascontiguousarray(preds[..., -1])


_NC = None


def _get_nc():
    global _NC
    if _NC is None:
        _NC = build_program()
    return _NC


def run(inputs, trace=False, **kw):
    nc = _get_nc()
    in_maps = make_in_maps(inputs)
    res = run_bass_kernel_spmd(nc, in_maps, list(range(CORES)), trace=trace, **kw)
    return assemble(res.results), res


def kernel(**inputs):
    (next_state, reward), _ = run(inputs, trace=False)
    return next_state, reward
